# revision 14
# baseline (speedup 1.0000x reference)
"""EquivariantAttention Trainium2 kernel.

Reference computation (B=4, S=512, D=512, H=8, DH=64):
    qkv = x @ W_qkv                      -> q, k, v  (b, s, h, dh)
    geo_w = geometric_features @ W_geo   -> (b, h, i, j)
    pos_w = positional_encodings @ W_pos -> (h, i, j)
    scores = q k^T / sqrt(dh) + geo_w + pos_w
    attn   = softmax_j(scores)            (mask is all-ones -> no-op)
    out    = (attn @ v) @ W_out

Sharding: the 512 MB positional_encodings table dominates HBM traffic, so the
query dim i is sharded across the 8 cores (64 rows each).  Every core computes
full k/v (cheap in bf16) and its own i-slice of the output; the host concats.

Within a core the scores live TRANSPOSED -- j on partitions, (h, i) in the
free dim -- because pos_w can only leave the tensor engine as out[M=j, N=h]
with d on the contraction partitions.  The host stages positional_encodings
pre-transposed to (d, i, j) and pre-cast to fp8-e4m3 (wpos stays bf16; the
PE takes mixed-dtype operands), so every tile lands partition-correct
straight off a plain HWDGE DMA at a quarter of the f32 byte count.

The i dim is processed in CH=32-row chunks that pipeline behind the P stream:
as soon as a chunk's slice of the table has been contracted into pos_w, that
chunk's scores/softmax/attn@v run while the next chunk is still streaming in.
This removes the serial [full 32 MB stream] -> [all attention compute]
dependency of the unchunked version.  Softmax over j (= partitions) skips
max-subtraction (scores are O(1)); the denominator is a matmul against a
ones-vector alongside the attn @ v matmuls.

`iters` unrolls the whole computation N times inside one NEFF (same inputs,
output rewritten) so the benchmark can amortize the multi-ms per-call
dispatch overhead of the axon/PJRT path; kernel() always uses iters=1.
"""

import numpy as np

B, S, D, H = 4, 512, 512, 8
DH = D // H            # 64
NCORES = 8
IS = S // NCORES       # 64  i-rows per core
T = B * S              # 2048 tokens
TI = B * IS            # 256 slice tokens
CH = 32                # i-rows per pipeline chunk
NCH = IS // CH         # 2 chunks
SUB = 16               # i-rows per P-stream DMA

_CACHE = {}


def _build_program(iters=1):
    from contextlib import ExitStack

    import concourse.bacc as bacc
    import concourse.mybir as mybir
    import concourse.tile as tile
    from concourse.masks import make_identity

    f32 = mybir.dt.float32
    bf16 = mybir.dt.bfloat16
    AF = mybir.ActivationFunctionType
    ALU = mybir.AluOpType

    nc = bacc.Bacc(
        "TRN2",
        target_bir_lowering=False,
        debug=False,
        enable_asserts=False,
        num_devices=NCORES,
    )

    x_d = nc.dram_tensor("x", [D, T], bf16, kind="ExternalInput").ap()
    xs_d = nc.dram_tensor("x_slice", [D, TI], bf16, kind="ExternalInput").ap()
    # positional_encodings arrive host-pre-transposed to (d, i, j) so the
    # contraction dim d lands on SBUF partitions straight out of the DMA
    f8 = mybir.dt.float8e4
    # fp8-e4m3 staging of the pos table: halves the dominant HBM stream
    # again (16 MB/core).  wpos stays bf16 -- the PE loads fp8 weights and
    # streams bf16, accumulating fp32.  Measured end-to-end error on the
    # fixed harness inputs: 1.3e-2 from this quantization (gate: 2e-2).
    p_d = nc.dram_tensor("pos_enc", [D, IS, S], f8, kind="ExternalInput").ap()
    g_d = nc.dram_tensor("geo", [B, 2, S, IS], bf16, kind="ExternalInput").ap()
    wqkv_d = nc.dram_tensor("w_qkv", [D, 3 * D], bf16, kind="ExternalInput").ap()
    wpos_d = nc.dram_tensor("w_pos", [D, H], bf16, kind="ExternalInput").ap()
    wgeo_d = nc.dram_tensor("w_geo", [2, H], f32, kind="ExternalInput").ap()
    wout_d = nc.dram_tensor("w_out", [D, D], bf16, kind="ExternalInput").ap()
    out_d = nc.dram_tensor("out", [B, IS, D], f32, kind="ExternalOutput").ap()
    out_flat = out_d.rearrange("b i d -> (b i) d")

    with tile.TileContext(nc) as tc, ExitStack() as ctx:
        # ---------------- static constants (hoisted out of the iter loop) --
        cp = ctx.enter_context(tc.tile_pool(name="consts", bufs=1))
        ident = cp.tile([128, 128], bf16, name="ident", tag="ident")
        make_identity(nc, ident)
        ones_col = cp.tile([128, 1], bf16, name="ones_col", tag="ones_col")
        nc.gpsimd.memset(ones_col, 1.0)
        ones_r128 = cp.tile([1, 128], bf16, name="ones_r128", tag="ones_r128")
        nc.gpsimd.memset(ones_r128, 1.0)

        # ---------------- persistent pools (slots recycle across iters) ----
        sb = ctx.enter_context(tc.tile_pool(name="sb", bufs=1))
        pt_pool = ctx.enter_context(tc.tile_pool(name="p_t", bufs=5))
        gt_pool = ctx.enter_context(tc.tile_pool(name="geoT", bufs=4))
        gtmp_pool = ctx.enter_context(tc.tile_pool(name="geo_tmp", bufs=2))
        att_pool = ctx.enter_context(tc.tile_pool(name="att_sb", bufs=3))
        fin_pool = ctx.enter_context(tc.tile_pool(name="fin", bufs=2))
        # PSUM: work 2 + pos 2 + o 2 + den 2 = 8 banks
        work_ps = ctx.enter_context(
            tc.tile_pool(name="work_ps", bufs=2, space="PSUM"))
        pos_pool = ctx.enter_context(
            tc.tile_pool(name="pos_ps", bufs=2, space="PSUM"))
        o_pool = ctx.enter_context(
            tc.tile_pool(name="o_ps", bufs=2, space="PSUM"))
        den_pool = ctx.enter_context(
            tc.tile_pool(name="den_ps", bufs=2, space="PSUM"))

        for _ in range(iters):
            _emit_iteration(
                nc, tc, mybir, sb, pt_pool, gt_pool, gtmp_pool, att_pool,
                fin_pool, work_ps, pos_pool, o_pool, den_pool,
                ident, ones_col, ones_r128,
                x_d, xs_d, p_d, g_d, wqkv_d, wpos_d, wgeo_d, wout_d, out_flat)

    nc.compile()
    return nc


def _emit_iteration(nc, tc, mybir, sb, pt_pool, gt_pool, gtmp_pool, att_pool,
                    fin_pool, work_ps, pos_pool, o_pool, den_pool,
                    ident, ones_col, ones_r128,
                    x_d, xs_d, p_d, g_d, wqkv_d, wpos_d, wgeo_d, wout_d,
                    out_flat):
    f32 = mybir.dt.float32
    bf16 = mybir.dt.bfloat16
    f8 = mybir.dt.float8e4
    AF = mybir.ActivationFunctionType
    ALU = mybir.AluOpType

    # ---------------- prologue DMAs: weights, x, geo (ACT ring) ----------
    wqkv_sb = []
    for dt_ in range(4):
        t_ = sb.tile([128, 3 * D], bf16, name=f"wqkv_{dt_}", tag=f"wqkv{dt_}")
        nc.scalar.dma_start(out=t_, in_=wqkv_d[dt_ * 128:(dt_ + 1) * 128, :])
        wqkv_sb.append(t_)
    wpos_sb = sb.tile([128, 32], bf16, name="wpos_sb", tag="wpos")
    nc.scalar.dma_start(
        out=wpos_sb.rearrange("p (a h) -> p a h", a=4),
        in_=wpos_d.rearrange("(a p) h -> p a h", p=128),
    )
    wgeo_flat = sb.tile([1, 16], f32, name="wgeo_flat", tag="wgf")
    nc.scalar.dma_start(
        out=wgeo_flat, in_=wgeo_d.rearrange("c h -> (c h)")[None, :])
    # W_geo broadcast to (128 j, (h, i64)) tiles so the pairwise-bias multiply
    # is one wide DVE op per (b, jb, c2) instead of 16 narrow ones
    wgeo_bc = []
    for c2 in range(2):
        wrep = sb.tile([1, 512], bf16, name=f"wrep{c2}", tag=f"wrep{c2}")
        nc.vector.tensor_copy(
            wrep.rearrange("p (h i) -> p h i", h=H),
            wgeo_flat[:, c2 * 8:(c2 + 1) * 8][:, :, None]
            .broadcast_to([1, H, IS]))
        bc_ps = work_ps.tile([128, 512], f32, name="bc_ps", tag="work")
        nc.tensor.matmul(bc_ps, ones_r128, wrep, start=True, stop=True)
        wb = sb.tile([128, 512], f32, name=f"wgbc{c2}", tag=f"wgbc{c2}")
        nc.vector.tensor_copy(wb, bc_ps)
        wgeo_bc.append(wb)

    wout_sb = []
    for db in range(4):
        t_ = sb.tile([128, D], bf16, name=f"wout_{db}", tag=f"wout{db}")
        nc.scalar.dma_start(out=t_, in_=wout_d[db * 128:(db + 1) * 128, :])
        wout_sb.append(t_)

    xT_sb = [sb.tile([128, T], bf16, name=f"xT_{db}", tag=f"xT{db}")
             for db in range(4)]
    xsT_sb = [sb.tile([128, TI], bf16, name=f"xsT_{db}", tag=f"xsT{db}")
              for db in range(4)]
    for db in range(4):
        nc.scalar.dma_start(out=xT_sb[db], in_=x_d[db * 128:(db + 1) * 128, :])
        nc.scalar.dma_start(out=xsT_sb[db], in_=xs_d[db * 128:(db + 1) * 128, :])

    g_nat = []
    for b in range(B):
        gn2 = []
        for c2 in range(2):
            # (128 j, (jb, i)) -- host pre-transposed, no PE transposes needed
            gn = sb.tile([128, 4 * IS], bf16, name=f"g_nat{b}_{c2}",
                         tag=f"gn{b}{c2}")
            nc.scalar.dma_start(
                out=gn.rearrange("p (a i) -> p a i", a=4),
                in_=g_d[b, c2].rearrange("(a p) i -> p a i", p=128))
            gn2.append(gn)
        g_nat.append(gn2)

    # ---------------- P-stream DMA issue helper (both HWDGE rings) -------
    ptg = {}           # (chunk, db, half) -> tile

    def issue_chunk_dmas(c):
        # half-major: matches the pos-matmul consumption order (il sweeps a
        # half across all db), so pool slots free in allocation order
        for half in range(CH // SUB):
            for db in range(4):
                pt = pt_pool.tile([128, SUB * 512], f8, name="ptg", tag="ptg")
                i0 = c * CH + half * SUB
                # all P-stream DMAs ride the SP HWDGE ring: the ACT ring
                # carries the prologue loads and ACT itself runs the exps,
                # so parking transfer time there would gate the softmax
                eng = nc.sync
                eng.dma_start(
                    out=pt.rearrange("p (a j) -> p a j", a=SUB),
                    in_=p_d[db * 128:(db + 1) * 128, i0:i0 + SUB, :],
                )
                ptg[(c, db, half)] = pt

    issue_chunk_dmas(0)

    # ---------------- projections: kT, v, qT (overlap chunk-0 stream) ----
    # k/q are stored HEAD-MAJOR in 64-partition tiles: every matmul operand
    # then has base_partition 0 (operands at base partition 64 hard-fault
    # the exec unit).
    kT_sb = [sb.tile([DH, T], bf16, name=f"kT_{h}", tag=f"kT{h}")
             for h in range(H)]
    v_sb = [sb.tile([128, D], bf16, name=f"v_{tt}", tag=f"v{tt}")
            for tt in range(T // 128)]
    qT_sb = [sb.tile([DH, TI], bf16, name=f"qT_{h}", tag=f"qT{h}")
             for h in range(H)]

    for h in range(H):
        ps = work_ps.tile([128, 512], f32, name="ps_q", tag="work")
        for dt_ in range(4):
            nc.tensor.matmul(
                ps[0:DH, 0:TI],
                wqkv_sb[dt_][:, h * DH:(h + 1) * DH],
                xsT_sb[dt_],
                start=(dt_ == 0), stop=(dt_ == 3),
            )
        nc.scalar.mul(qT_sb[h], ps[0:DH, 0:TI], 0.125)   # fold 1/sqrt(DH)

    for h in range(H):
        for tch in range(4):
            ps = work_ps.tile([128, 512], f32, name="ps_k", tag="work")
            for dt_ in range(4):
                nc.tensor.matmul(
                    ps[0:DH, :],
                    wqkv_sb[dt_][:, 512 + h * DH: 512 + (h + 1) * DH],
                    xT_sb[dt_][:, tch * 512:(tch + 1) * 512],
                    start=(dt_ == 0), stop=(dt_ == 3),
                )
            dst = kT_sb[h][:, tch * 512:(tch + 1) * 512]
            if (h + tch) % 2 == 0:
                nc.vector.tensor_copy(dst, ps[0:DH, :])
            else:
                nc.scalar.copy(dst, ps[0:DH, :])

    for tt in range(T // 128):
        ps = work_ps.tile([128, 512], f32, name="ps_v", tag="work")
        for dt_ in range(4):
            nc.tensor.matmul(
                ps,
                xT_sb[dt_][:, tt * 128:(tt + 1) * 128],
                wqkv_sb[dt_][:, 1024:1536],
                start=(dt_ == 0), stop=(dt_ == 3),
            )
        if tt % 2 == 0:
            nc.vector.tensor_copy(v_sb[tt], ps)
        else:
            nc.scalar.copy(v_sb[tt], ps)

    # ---------------- geo biases -> exp(geo_w), (b, jb) tiles ------------
    # geo_exp[b][jb] : (128 j, (h, i64)) bf16, col = h*64 + i
    geo_exp = [[sb.tile([128, 512], bf16, name=f"geoexp_{b}_{jb}",
                        tag=f"gx{b}{jb}", bufs=2) for jb in range(4)]
               for b in range(B)]
    for b in range(B):
        for jb in range(4):
            g0 = (g_nat[b][0][:, jb * IS:(jb + 1) * IS][:, None, :]
                  .broadcast_to([128, H, IS]))
            g1 = (g_nat[b][1][:, jb * IS:(jb + 1) * IS][:, None, :]
                  .broadcast_to([128, H, IS]))
            m0 = gtmp_pool.tile([128, 512], f32, name="m0", tag="m0")
            nc.vector.tensor_mul(
                m0.rearrange("p (h i) -> p h i", h=H), g0,
                wgeo_bc[0].rearrange("p (h i) -> p h i", h=H))
            m1 = gtmp_pool.tile([128, 512], f32, name="m1", tag="m1")
            nc.vector.tensor_mul(
                m1.rearrange("p (h i) -> p h i", h=H), g1,
                wgeo_bc[1].rearrange("p (h i) -> p h i", h=H))
            gtmp = gtmp_pool.tile([128, 512], f32, name="gtmp", tag="gtmp")
            nc.vector.tensor_add(gtmp, m0, m1)
            nc.scalar.activation(geo_exp[b][jb], gtmp, AF.Exp)

    # ---------------- chunk pipeline: pos_w -> scores -> attn @ v --------
    O_sb = [sb.tile([IS, D], bf16, name=f"O_{b}", tag=f"O{b}")
            for b in range(B)]

    for c in range(NCH):
        if c + 1 < NCH:
            issue_chunk_dmas(c + 1)

        # pos_w for this chunk: two PSUM banks (jb pair each),
        # col within bank = (jb%2)*256 + i_local*8 + h
        pos_ps = [pos_pool.tile([128, 512], f32, name="pos_ps", tag="pos")
                  for _ in range(2)]
        for il in range(CH):
            half, ir = il // SUB, il % SUB
            for db in range(4):
                pt = ptg[(c, db, half)]
                for jb in range(4):
                    base = (jb % 2) * 256 + il * 8
                    nc.tensor.matmul(
                        pos_ps[jb // 2][:, base: base + 8],
                        pt[:, ir * 512 + jb * 128: ir * 512 + (jb + 1) * 128],
                        wpos_sb[:, db * 8:(db + 1) * 8],
                        start=(il == 0 and db == 0 and jb % 2 == 0),
                        stop=(il == CH - 1 and db == 3 and jb % 2 == 1),
                    )
        # re-layout (i,h) -> (h,i) while copying PSUM -> SBUF, per jb
        pos_sb = []
        for jb in range(4):
            psb = att_pool.tile([128, 256], f32, name="pos_sb",
                                tag=f"possb{jb}", bufs=2)
            src_ap = (pos_ps[jb // 2][:, (jb % 2) * 256:(jb % 2) * 256 + 256]
                      .rearrange("p (i h) -> p h i", h=H))
            dst_ap = psb.rearrange("p (h i) -> p h i", h=H)
            if jb % 2 == 0:
                nc.vector.tensor_copy(dst_ap, src_ap)
            else:
                nc.scalar.copy(dst_ap, src_ap)
            pos_sb.append(psb)

        for b in range(B):
            o_ps = o_pool.tile([CH, 512], f32, name="o_ps", tag="o")
            den_ps = den_pool.tile([CH, H], f32, name="den_ps", tag="den")
            ex_tiles = [None] * 4

            def emit_qk(jb):
                qk = work_ps.tile([128, 512], f32, name="qk", tag="work")
                for h in range(H):
                    nc.tensor.matmul(
                        qk[:, h * CH:(h + 1) * CH],
                        kT_sb[h][:, b * S + jb * 128: b * S + (jb + 1) * 128],
                        qT_sb[h][:, b * IS + c * CH: b * IS + (c + 1) * CH],
                        start=(h == 0), stop=(h == H - 1),
                    )
                return qk

            def emit_softmax(jb, qk):
                t1 = att_pool.tile([128, 256], f32, name="t1", tag="t1")
                nc.vector.tensor_add(t1, qk[:, 0:256], pos_sb[jb])
                e1 = att_pool.tile([128, 256], bf16, name="e1", tag="e1")
                nc.scalar.activation(e1, t1, AF.Exp)
                ex = att_pool.tile([128, 256], bf16, name="ex", tag="ex",
                                   bufs=4)
                gv = (geo_exp[b][jb]
                      .rearrange("p (h i) -> p h i", h=H)
                      [:, :, c * CH:(c + 1) * CH])
                nc.vector.tensor_mul(
                    ex.rearrange("p (h i) -> p h i", h=H),
                    e1.rearrange("p (h i) -> p h i", h=H), gv)
                return ex

            def emit_av(jb, ex):
                tt = b * 4 + jb
                for h in range(H):
                    lhs = ex[:, h * CH:(h + 1) * CH]
                    nc.tensor.matmul(
                        o_ps[:, h * DH:(h + 1) * DH],
                        lhs, v_sb[tt][:, h * DH:(h + 1) * DH],
                        start=(jb == 0 and h == 0),
                        stop=(jb == 3 and h == H - 1),
                    )
                    nc.tensor.matmul(
                        den_ps[:, h:h + 1], lhs, ones_col,
                        start=(jb == 0 and h == 0),
                        stop=(jb == 3 and h == H - 1),
                    )

            # software-pipeline: qk(jb+1) issues before av(jb) so the PE
            # fills the softmax (DVE/ACT) latency with useful matmuls
            qk = emit_qk(0)
            for jb in range(4):
                ex_tiles[jb] = emit_softmax(jb, qk)
                if jb + 1 < 4:
                    qk = emit_qk(jb + 1)
                emit_av(jb, ex_tiles[jb])

            recip = att_pool.tile([CH, H], f32, name="recip", tag="recip")
            nc.vector.reciprocal(recip, den_ps)
            for h in range(H):
                nc.vector.tensor_scalar(
                    O_sb[b][c * CH:(c + 1) * CH, h * DH:(h + 1) * DH],
                    o_ps[:, h * DH:(h + 1) * DH],
                    recip[:, h:h + 1], None, op0=ALU.mult)

    # ---------------- epilogue: out = O @ W_out --------------------------
    otT = [fin_pool.tile([128, TI], bf16, name=f"otT_{db}", tag=f"otT{db}",
                         bufs=1)
           for db in range(4)]
    for db in range(4):
        tr_ps = pos_pool.tile([128, 512], bf16, name="tr_ps", tag="pos")
        for b in range(B):
            nc.tensor.transpose(
                tr_ps[:, b * IS:(b + 1) * IS],
                O_sb[b][:, db * 128:(db + 1) * 128],
                ident[0:IS, 0:IS],
            )
        nc.vector.tensor_copy(otT[db], tr_ps[:, 0:TI])
    for tt2 in range(TI // 128):
        f_ps = o_pool.tile([128, 512], f32, name="f_ps", tag="o")
        for db in range(4):
            nc.tensor.matmul(
                f_ps,
                otT[db][:, tt2 * 128:(tt2 + 1) * 128],
                wout_sb[db],
                start=(db == 0), stop=(db == 3),
            )
        fout = fin_pool.tile([128, D], f32, name="fout", tag="fout", bufs=2)
        nc.vector.tensor_copy(fout, f_ps)
        nc.sync.dma_start(
            out=out_flat[tt2 * 128:(tt2 + 1) * 128, :], in_=fout)


def _get_program():
    if "nc" not in _CACHE:
        _CACHE["nc"] = _build_program(iters=1)
    return _CACHE["nc"]


def make_in_maps(inputs):
    import ml_dtypes
    bf = ml_dtypes.bfloat16
    x = np.asarray(inputs["x"], np.float32)                       # (B, S, D)
    geo = np.asarray(inputs["geometric_features"], np.float32)    # (B, S, S, 2)
    pos = np.asarray(inputs["positional_encodings"], np.float32)  # (S, S, D)
    wqkv = np.asarray(inputs["W_qkv"], np.float32)
    wout = np.asarray(inputs["W_out"], np.float32)
    wgeo = np.asarray(inputs["W_geo"], np.float32)
    wpos = np.asarray(inputs["W_pos"], np.float32)
    mask = np.asarray(inputs["mask"])

    assert mask.all(), "kernel assumes an all-true mask"
    for k in ("b_qkv", "b_out", "b_geo", "b_pos"):
        assert not np.asarray(inputs[k], np.float32).any(), \
            "kernel assumes zero biases (reference setup_inputs uses zeros)"

    # big inputs staged as bf16 on the host: halves device HBM traffic and
    # makes every load a plain HWDGE DMA (matmuls consume bf16 anyway)
    x_flat = np.ascontiguousarray(x.reshape(T, D).T.astype(bf))
    wqkv_b = np.ascontiguousarray(wqkv.astype(bf))
    wpos_b = np.ascontiguousarray(wpos.astype(bf))
    wout_b = np.ascontiguousarray(wout.astype(bf))
    in_maps = []
    for c in range(NCORES):
        lo = c * IS
        in_maps.append({
            "x": x_flat,
            "x_slice": np.ascontiguousarray(
                x[:, lo:lo + IS].reshape(TI, D).T.astype(bf)),
            "pos_enc": np.ascontiguousarray(
                pos[lo:lo + IS].transpose(2, 0, 1)
                .astype(ml_dtypes.float8_e4m3)),
            "geo": np.ascontiguousarray(
                geo[:, lo:lo + IS].transpose(0, 3, 2, 1).astype(bf)),
            "w_qkv": wqkv_b,
            "w_pos": wpos_b,
            "w_geo": wgeo,
            "w_out": wout_b,
        })
    return in_maps


def gather_out(results):
    out = np.empty((B, S, D), np.float32)
    for c in range(NCORES):
        out[:, c * IS:(c + 1) * IS, :] = results[c]["out"]
    return out


def kernel(**inputs) -> np.ndarray:
    from concourse.bass_utils import run_bass_kernel_spmd

    nc = _get_program()
    in_maps = make_in_maps(inputs)
    res = run_bass_kernel_spmd(nc, in_maps, core_ids=list(range(NCORES)))
    return gather_out(res.results)


# revision 15
# speedup vs baseline: 1.2130x; 1.2130x over previous
"""EquivariantAttention Trainium2 kernel.

Reference computation (B=4, S=512, D=512, H=8, DH=64):
    qkv = x @ W_qkv                      -> q, k, v  (b, s, h, dh)
    geo_w = geometric_features @ W_geo   -> (b, h, i, j)
    pos_w = positional_encodings @ W_pos -> (h, i, j)
    scores = q k^T / sqrt(dh) + geo_w + pos_w
    attn   = softmax_j(scores)            (mask is all-ones -> no-op)
    out    = (attn @ v) @ W_out

Sharding: the 512 MB positional_encodings table dominates HBM traffic, so the
query dim i is sharded across the 8 cores (64 rows each).  Every core computes
full k/v (cheap in bf16) and its own i-slice of the output; the host concats.

Within a core the scores live TRANSPOSED -- j on partitions, (h, i) in the
free dim -- because pos_w can only leave the tensor engine as out[M=j, N=h]
with d on the contraction partitions.  The host stages positional_encodings
pre-transposed to (d, i, j) and pre-cast to fp8-e4m3 (wpos stays bf16; the
PE takes mixed-dtype operands), so every tile lands partition-correct
straight off a plain HWDGE DMA at a quarter of the f32 byte count.

The i dim is processed in CH=32-row chunks that pipeline behind the P stream:
as soon as a chunk's slice of the table has been contracted into pos_w, that
chunk's scores/softmax/attn@v run while the next chunk is still streaming in.
This removes the serial [full 32 MB stream] -> [all attention compute]
dependency of the unchunked version.  Softmax over j (= partitions) skips
max-subtraction (scores are O(1)); the denominator is a matmul against a
ones-vector alongside the attn @ v matmuls.

`iters` unrolls the whole computation N times inside one NEFF (same inputs,
output rewritten) so the benchmark can amortize the multi-ms per-call
dispatch overhead of the axon/PJRT path; kernel() always uses iters=1.
"""

import numpy as np

B, S, D, H = 4, 512, 512, 8
DH = D // H            # 64
NCORES = 8
IS = S // NCORES       # 64  i-rows per core
T = B * S              # 2048 tokens
TI = B * IS            # 256 slice tokens
CH = 32                # i-rows per pipeline chunk
NCH = IS // CH         # 2 chunks
SUB = 8                # i-rows per P-stream DMA

_CACHE = {}


def _build_program(iters=1):
    from contextlib import ExitStack

    import concourse.bacc as bacc
    import concourse.mybir as mybir
    import concourse.tile as tile
    from concourse.masks import make_identity

    f32 = mybir.dt.float32
    bf16 = mybir.dt.bfloat16
    AF = mybir.ActivationFunctionType
    ALU = mybir.AluOpType

    nc = bacc.Bacc(
        "TRN2",
        target_bir_lowering=False,
        debug=False,
        enable_asserts=False,
        num_devices=NCORES,
    )

    x_d = nc.dram_tensor("x", [D, T], bf16, kind="ExternalInput").ap()
    xs_d = nc.dram_tensor("x_slice", [D, TI], bf16, kind="ExternalInput").ap()
    # positional_encodings arrive host-pre-transposed to (d, i, j) so the
    # contraction dim d lands on SBUF partitions straight out of the DMA
    f8 = mybir.dt.float8e4
    # fp8-e4m3 staging of the pos table: halves the dominant HBM stream
    # again (16 MB/core).  wpos stays bf16 -- the PE loads fp8 weights and
    # streams bf16, accumulating fp32.  Measured end-to-end error on the
    # fixed harness inputs: 1.3e-2 from this quantization (gate: 2e-2).
    p_d = nc.dram_tensor("pos_enc", [D, IS, S], f8, kind="ExternalInput").ap()
    g_d = nc.dram_tensor("geo", [B, 2, S, IS], bf16, kind="ExternalInput").ap()
    wqkv_d = nc.dram_tensor("w_qkv", [D, 3 * D], bf16, kind="ExternalInput").ap()
    wpos_d = nc.dram_tensor("w_pos", [D, H], bf16, kind="ExternalInput").ap()
    wgeo_d = nc.dram_tensor("w_geo", [2, H], f32, kind="ExternalInput").ap()
    wout_d = nc.dram_tensor("w_out", [D, D], bf16, kind="ExternalInput").ap()
    out_d = nc.dram_tensor("out", [B, IS, D], f32, kind="ExternalOutput").ap()
    out_flat = out_d.rearrange("b i d -> (b i) d")

    with tile.TileContext(nc) as tc, ExitStack() as ctx:
        # ---------------- static constants (hoisted out of the iter loop) --
        cp = ctx.enter_context(tc.tile_pool(name="consts", bufs=1))
        ident = cp.tile([128, 128], bf16, name="ident", tag="ident")
        make_identity(nc, ident)
        ones_col = cp.tile([128, 1], bf16, name="ones_col", tag="ones_col")
        nc.gpsimd.memset(ones_col, 1.0)
        ones_r128 = cp.tile([1, 128], bf16, name="ones_r128", tag="ones_r128")
        nc.gpsimd.memset(ones_r128, 1.0)

        # ---------------- persistent pools (slots recycle across iters) ----
        sb = ctx.enter_context(tc.tile_pool(name="sb", bufs=1))
        pt_pool = ctx.enter_context(tc.tile_pool(name="p_t", bufs=8))
        gt_pool = ctx.enter_context(tc.tile_pool(name="geoT", bufs=4))
        gtmp_pool = ctx.enter_context(tc.tile_pool(name="geo_tmp", bufs=2))
        att_pool = ctx.enter_context(tc.tile_pool(name="att_sb", bufs=3))
        fin_pool = ctx.enter_context(tc.tile_pool(name="fin", bufs=2))
        # PSUM: work 2 + pos 2 + o 2 + den 2 = 8 banks
        work_ps = ctx.enter_context(
            tc.tile_pool(name="work_ps", bufs=2, space="PSUM"))
        pos_pool = ctx.enter_context(
            tc.tile_pool(name="pos_ps", bufs=2, space="PSUM"))
        o_pool = ctx.enter_context(
            tc.tile_pool(name="o_ps", bufs=2, space="PSUM"))
        den_pool = ctx.enter_context(
            tc.tile_pool(name="den_ps", bufs=2, space="PSUM"))

        for _ in range(iters):
            _emit_iteration(
                nc, tc, mybir, sb, pt_pool, gt_pool, gtmp_pool, att_pool,
                fin_pool, work_ps, pos_pool, o_pool, den_pool,
                ident, ones_col, ones_r128,
                x_d, xs_d, p_d, g_d, wqkv_d, wpos_d, wgeo_d, wout_d, out_flat)

    nc.compile()
    return nc


def _emit_iteration(nc, tc, mybir, sb, pt_pool, gt_pool, gtmp_pool, att_pool,
                    fin_pool, work_ps, pos_pool, o_pool, den_pool,
                    ident, ones_col, ones_r128,
                    x_d, xs_d, p_d, g_d, wqkv_d, wpos_d, wgeo_d, wout_d,
                    out_flat):
    f32 = mybir.dt.float32
    bf16 = mybir.dt.bfloat16
    f8 = mybir.dt.float8e4
    AF = mybir.ActivationFunctionType
    ALU = mybir.AluOpType

    # ---------------- prologue DMAs: weights, x, geo (ACT ring) ----------
    wqkv_sb = []
    for dt_ in range(4):
        t_ = sb.tile([128, 3 * D], bf16, name=f"wqkv_{dt_}", tag=f"wqkv{dt_}")
        nc.scalar.dma_start(out=t_, in_=wqkv_d[dt_ * 128:(dt_ + 1) * 128, :])
        wqkv_sb.append(t_)
    wpos_sb = sb.tile([128, 32], bf16, name="wpos_sb", tag="wpos")
    nc.scalar.dma_start(
        out=wpos_sb.rearrange("p (a h) -> p a h", a=4),
        in_=wpos_d.rearrange("(a p) h -> p a h", p=128),
    )
    wgeo_flat = sb.tile([1, 16], f32, name="wgeo_flat", tag="wgf")
    nc.scalar.dma_start(
        out=wgeo_flat, in_=wgeo_d.rearrange("c h -> (c h)")[None, :])
    # W_geo broadcast to (128 j, (h, i64)) tiles so the pairwise-bias multiply
    # is one wide DVE op per (b, jb, c2) instead of 16 narrow ones
    wgeo_bc = []
    for c2 in range(2):
        wrep = sb.tile([1, 512], bf16, name=f"wrep{c2}", tag=f"wrep{c2}")
        nc.vector.tensor_copy(
            wrep.rearrange("p (h i) -> p h i", h=H),
            wgeo_flat[:, c2 * 8:(c2 + 1) * 8][:, :, None]
            .broadcast_to([1, H, IS]))
        bc_ps = work_ps.tile([128, 512], f32, name="bc_ps", tag="work")
        nc.tensor.matmul(bc_ps, ones_r128, wrep, start=True, stop=True)
        wb = sb.tile([128, 512], f32, name=f"wgbc{c2}", tag=f"wgbc{c2}")
        nc.vector.tensor_copy(wb, bc_ps)
        wgeo_bc.append(wb)

    wout_sb = []
    for db in range(4):
        t_ = sb.tile([128, D], bf16, name=f"wout_{db}", tag=f"wout{db}")
        nc.scalar.dma_start(out=t_, in_=wout_d[db * 128:(db + 1) * 128, :])
        wout_sb.append(t_)

    xT_sb = [sb.tile([128, T], bf16, name=f"xT_{db}", tag=f"xT{db}")
             for db in range(4)]
    xsT_sb = [sb.tile([128, TI], bf16, name=f"xsT_{db}", tag=f"xsT{db}")
              for db in range(4)]
    for db in range(4):
        nc.scalar.dma_start(out=xT_sb[db], in_=x_d[db * 128:(db + 1) * 128, :])
        nc.scalar.dma_start(out=xsT_sb[db], in_=xs_d[db * 128:(db + 1) * 128, :])

    g_nat = []
    for b in range(B):
        gn2 = []
        for c2 in range(2):
            # (128 j, (jb, i)) -- host pre-transposed, no PE transposes needed
            gn = sb.tile([128, 4 * IS], bf16, name=f"g_nat{b}_{c2}",
                         tag=f"gn{b}{c2}")
            nc.scalar.dma_start(
                out=gn.rearrange("p (a i) -> p a i", a=4),
                in_=g_d[b, c2].rearrange("(a p) i -> p a i", p=128))
            gn2.append(gn)
        g_nat.append(gn2)

    # ---------------- P-stream DMA issue helper (both HWDGE rings) -------
    ptg = {}           # (chunk, db, half) -> tile

    def issue_chunk_dmas(c):
        # half-major: matches the pos-matmul consumption order (il sweeps a
        # half across all db), so pool slots free in allocation order
        for half in range(CH // SUB):
            for db in range(4):
                pt = pt_pool.tile([128, SUB * 512], f8, name="ptg", tag="ptg")
                i0 = c * CH + half * SUB
                # all P-stream DMAs ride the SP HWDGE ring: the ACT ring
                # carries the prologue loads and ACT itself runs the exps,
                # so parking transfer time there would gate the softmax
                eng = nc.sync
                eng.dma_start(
                    out=pt.rearrange("p (a j) -> p a j", a=SUB),
                    in_=p_d[db * 128:(db + 1) * 128, i0:i0 + SUB, :],
                )
                ptg[(c, db, half)] = pt

    issue_chunk_dmas(0)

    # ---------------- projections: kT, v, qT (overlap chunk-0 stream) ----
    # k/q are stored HEAD-MAJOR in 64-partition tiles: every matmul operand
    # then has base_partition 0 (operands at base partition 64 hard-fault
    # the exec unit).
    kT_sb = [sb.tile([DH, T], bf16, name=f"kT_{h}", tag=f"kT{h}")
             for h in range(H)]
    v_sb = [sb.tile([128, D], bf16, name=f"v_{tt}", tag=f"v{tt}")
            for tt in range(T // 128)]
    qT_sb = [sb.tile([DH, TI], bf16, name=f"qT_{h}", tag=f"qT{h}")
             for h in range(H)]

    for h in range(H):
        ps = work_ps.tile([128, 512], f32, name="ps_q", tag="work")
        for dt_ in range(4):
            nc.tensor.matmul(
                ps[0:DH, 0:TI],
                wqkv_sb[dt_][:, h * DH:(h + 1) * DH],
                xsT_sb[dt_],
                start=(dt_ == 0), stop=(dt_ == 3),
            )
        nc.scalar.mul(qT_sb[h], ps[0:DH, 0:TI], 0.125)   # fold 1/sqrt(DH)

    for h in range(H):
        for tch in range(4):
            ps = work_ps.tile([128, 512], f32, name="ps_k", tag="work")
            for dt_ in range(4):
                nc.tensor.matmul(
                    ps[0:DH, :],
                    wqkv_sb[dt_][:, 512 + h * DH: 512 + (h + 1) * DH],
                    xT_sb[dt_][:, tch * 512:(tch + 1) * 512],
                    start=(dt_ == 0), stop=(dt_ == 3),
                )
            dst = kT_sb[h][:, tch * 512:(tch + 1) * 512]
            if (h + tch) % 2 == 0:
                nc.vector.tensor_copy(dst, ps[0:DH, :])
            else:
                nc.scalar.copy(dst, ps[0:DH, :])

    for tt in range(T // 128):
        ps = work_ps.tile([128, 512], f32, name="ps_v", tag="work")
        for dt_ in range(4):
            nc.tensor.matmul(
                ps,
                xT_sb[dt_][:, tt * 128:(tt + 1) * 128],
                wqkv_sb[dt_][:, 1024:1536],
                start=(dt_ == 0), stop=(dt_ == 3),
            )
        if tt % 2 == 0:
            nc.vector.tensor_copy(v_sb[tt], ps)
        else:
            nc.scalar.copy(v_sb[tt], ps)

    # ---------------- geo biases -> exp(geo_w), (b, jb) tiles ------------
    # geo_exp[b][jb] : (128 j, (h, i64)) bf16, col = h*64 + i
    geo_exp = [[sb.tile([128, 512], bf16, name=f"geoexp_{b}_{jb}",
                        tag=f"gx{b}{jb}", bufs=2) for jb in range(4)]
               for b in range(B)]
    for b in range(B):
        for jb in range(4):
            g0 = (g_nat[b][0][:, jb * IS:(jb + 1) * IS][:, None, :]
                  .broadcast_to([128, H, IS]))
            g1 = (g_nat[b][1][:, jb * IS:(jb + 1) * IS][:, None, :]
                  .broadcast_to([128, H, IS]))
            m0 = gtmp_pool.tile([128, 512], f32, name="m0", tag="m0")
            nc.vector.tensor_mul(
                m0.rearrange("p (h i) -> p h i", h=H), g0,
                wgeo_bc[0].rearrange("p (h i) -> p h i", h=H))
            m1 = gtmp_pool.tile([128, 512], f32, name="m1", tag="m1")
            nc.vector.tensor_mul(
                m1.rearrange("p (h i) -> p h i", h=H), g1,
                wgeo_bc[1].rearrange("p (h i) -> p h i", h=H))
            gtmp = gtmp_pool.tile([128, 512], f32, name="gtmp", tag="gtmp")
            nc.vector.tensor_add(gtmp, m0, m1)
            nc.scalar.activation(geo_exp[b][jb], gtmp, AF.Exp)

    # ---------------- chunk pipeline: pos_w -> scores -> attn @ v --------
    O_sb = [sb.tile([IS, D], bf16, name=f"O_{b}", tag=f"O{b}")
            for b in range(B)]

    for c in range(NCH):
        if c + 1 < NCH:
            issue_chunk_dmas(c + 1)

        # pos_w for this chunk: two PSUM banks (jb pair each),
        # col within bank = (jb%2)*256 + i_local*8 + h
        pos_ps = [pos_pool.tile([128, 512], f32, name="pos_ps", tag="pos")
                  for _ in range(2)]
        for il in range(CH):
            half, ir = il // SUB, il % SUB
            for db in range(4):
                pt = ptg[(c, db, half)]
                for jb in range(4):
                    base = (jb % 2) * 256 + il * 8
                    nc.tensor.matmul(
                        pos_ps[jb // 2][:, base: base + 8],
                        pt[:, ir * 512 + jb * 128: ir * 512 + (jb + 1) * 128],
                        wpos_sb[:, db * 8:(db + 1) * 8],
                        start=(il == 0 and db == 0 and jb % 2 == 0),
                        stop=(il == CH - 1 and db == 3 and jb % 2 == 1),
                    )
        # re-layout (i,h) -> (h,i) while copying PSUM -> SBUF, per jb
        pos_sb = []
        for jb in range(4):
            psb = att_pool.tile([128, 256], f32, name="pos_sb",
                                tag=f"possb{jb}", bufs=2)
            src_ap = (pos_ps[jb // 2][:, (jb % 2) * 256:(jb % 2) * 256 + 256]
                      .rearrange("p (i h) -> p h i", h=H))
            dst_ap = psb.rearrange("p (h i) -> p h i", h=H)
            if jb % 2 == 0:
                nc.vector.tensor_copy(dst_ap, src_ap)
            else:
                nc.scalar.copy(dst_ap, src_ap)
            pos_sb.append(psb)

        for b in range(B):
            o_ps = o_pool.tile([CH, 512], f32, name="o_ps", tag="o")
            den_ps = den_pool.tile([CH, H], f32, name="den_ps", tag="den")
            ex_tiles = [None] * 4

            def emit_qk(jb):
                qk = work_ps.tile([128, 512], f32, name="qk", tag="work")
                for h in range(H):
                    nc.tensor.matmul(
                        qk[:, h * CH:(h + 1) * CH],
                        kT_sb[h][:, b * S + jb * 128: b * S + (jb + 1) * 128],
                        qT_sb[h][:, b * IS + c * CH: b * IS + (c + 1) * CH],
                        start=(h == 0), stop=(h == H - 1),
                    )
                return qk

            def emit_softmax(jb, qk):
                t1 = att_pool.tile([128, 256], f32, name="t1", tag="t1")
                nc.vector.tensor_add(t1, qk[:, 0:256], pos_sb[jb])
                e1 = att_pool.tile([128, 256], bf16, name="e1", tag="e1")
                nc.scalar.activation(e1, t1, AF.Exp)
                ex = att_pool.tile([128, 256], bf16, name="ex", tag="ex",
                                   bufs=4)
                gv = (geo_exp[b][jb]
                      .rearrange("p (h i) -> p h i", h=H)
                      [:, :, c * CH:(c + 1) * CH])
                nc.vector.tensor_mul(
                    ex.rearrange("p (h i) -> p h i", h=H),
                    e1.rearrange("p (h i) -> p h i", h=H), gv)
                return ex

            def emit_av(jb, ex):
                tt = b * 4 + jb
                for h in range(H):
                    lhs = ex[:, h * CH:(h + 1) * CH]
                    nc.tensor.matmul(
                        o_ps[:, h * DH:(h + 1) * DH],
                        lhs, v_sb[tt][:, h * DH:(h + 1) * DH],
                        start=(jb == 0 and h == 0),
                        stop=(jb == 3 and h == H - 1),
                    )
                    nc.tensor.matmul(
                        den_ps[:, h:h + 1], lhs, ones_col,
                        start=(jb == 0 and h == 0),
                        stop=(jb == 3 and h == H - 1),
                    )

            # software-pipeline: qk(jb+1) issues before av(jb) so the PE
            # fills the softmax (DVE/ACT) latency with useful matmuls
            qk = emit_qk(0)
            for jb in range(4):
                ex_tiles[jb] = emit_softmax(jb, qk)
                if jb + 1 < 4:
                    qk = emit_qk(jb + 1)
                emit_av(jb, ex_tiles[jb])

            recip = att_pool.tile([CH, H], f32, name="recip", tag="recip")
            nc.vector.reciprocal(recip, den_ps)
            for h in range(H):
                nc.vector.tensor_scalar(
                    O_sb[b][c * CH:(c + 1) * CH, h * DH:(h + 1) * DH],
                    o_ps[:, h * DH:(h + 1) * DH],
                    recip[:, h:h + 1], None, op0=ALU.mult)

    # ---------------- epilogue: out = O @ W_out --------------------------
    otT = [fin_pool.tile([128, TI], bf16, name=f"otT_{db}", tag=f"otT{db}",
                         bufs=1)
           for db in range(4)]
    for db in range(4):
        tr_ps = pos_pool.tile([128, 512], bf16, name="tr_ps", tag="pos")
        for b in range(B):
            nc.tensor.transpose(
                tr_ps[:, b * IS:(b + 1) * IS],
                O_sb[b][:, db * 128:(db + 1) * 128],
                ident[0:IS, 0:IS],
            )
        nc.vector.tensor_copy(otT[db], tr_ps[:, 0:TI])
    for tt2 in range(TI // 128):
        f_ps = o_pool.tile([128, 512], f32, name="f_ps", tag="o")
        for db in range(4):
            nc.tensor.matmul(
                f_ps,
                otT[db][:, tt2 * 128:(tt2 + 1) * 128],
                wout_sb[db],
                start=(db == 0), stop=(db == 3),
            )
        fout = fin_pool.tile([128, D], f32, name="fout", tag="fout", bufs=2)
        nc.vector.tensor_copy(fout, f_ps)
        nc.sync.dma_start(
            out=out_flat[tt2 * 128:(tt2 + 1) * 128, :], in_=fout)


def _get_program():
    if "nc" not in _CACHE:
        _CACHE["nc"] = _build_program(iters=1)
    return _CACHE["nc"]


def make_in_maps(inputs):
    import ml_dtypes
    bf = ml_dtypes.bfloat16
    x = np.asarray(inputs["x"], np.float32)                       # (B, S, D)
    geo = np.asarray(inputs["geometric_features"], np.float32)    # (B, S, S, 2)
    pos = np.asarray(inputs["positional_encodings"], np.float32)  # (S, S, D)
    wqkv = np.asarray(inputs["W_qkv"], np.float32)
    wout = np.asarray(inputs["W_out"], np.float32)
    wgeo = np.asarray(inputs["W_geo"], np.float32)
    wpos = np.asarray(inputs["W_pos"], np.float32)
    mask = np.asarray(inputs["mask"])

    assert mask.all(), "kernel assumes an all-true mask"
    for k in ("b_qkv", "b_out", "b_geo", "b_pos"):
        assert not np.asarray(inputs[k], np.float32).any(), \
            "kernel assumes zero biases (reference setup_inputs uses zeros)"

    # big inputs staged as bf16 on the host: halves device HBM traffic and
    # makes every load a plain HWDGE DMA (matmuls consume bf16 anyway)
    x_flat = np.ascontiguousarray(x.reshape(T, D).T.astype(bf))
    wqkv_b = np.ascontiguousarray(wqkv.astype(bf))
    wpos_b = np.ascontiguousarray(wpos.astype(bf))
    wout_b = np.ascontiguousarray(wout.astype(bf))
    in_maps = []
    for c in range(NCORES):
        lo = c * IS
        in_maps.append({
            "x": x_flat,
            "x_slice": np.ascontiguousarray(
                x[:, lo:lo + IS].reshape(TI, D).T.astype(bf)),
            "pos_enc": np.ascontiguousarray(
                pos[lo:lo + IS].transpose(2, 0, 1)
                .astype(ml_dtypes.float8_e4m3)),
            "geo": np.ascontiguousarray(
                geo[:, lo:lo + IS].transpose(0, 3, 2, 1).astype(bf)),
            "w_qkv": wqkv_b,
            "w_pos": wpos_b,
            "w_geo": wgeo,
            "w_out": wout_b,
        })
    return in_maps


def gather_out(results):
    out = np.empty((B, S, D), np.float32)
    for c in range(NCORES):
        out[:, c * IS:(c + 1) * IS, :] = results[c]["out"]
    return out


def kernel(**inputs) -> np.ndarray:
    from concourse.bass_utils import run_bass_kernel_spmd

    nc = _get_program()
    in_maps = make_in_maps(inputs)
    res = run_bass_kernel_spmd(nc, in_maps, core_ids=list(range(NCORES)))
    return gather_out(res.results)
